# revision 7
# baseline (speedup 1.0000x reference)
"""Trainium2 Bass kernel for weighted Chamfer loss.

Problem: B=4 batches of N=8192 3-D points (pred, gt) + per-point weights.
loss = mean_{b,n}[ (min_m d2(p_n,g_m) + min_m d2(g_n,p_m)) * mean(weight) ]

Strategy (8 NeuronCores):
  - Shard (batch, half): core c -> batch c//2, point-half c%2. Each core
    computes, for its 4096 points, the exact min squared distance to all
    8192 opposite points, in BOTH directions (pred->gt for its pred half,
    gt->pred for its gt half). The host just sums the 8 per-core weighted
    partial sums -- no cross-core combining.
  - On-core: distance matrices come from the TensorEngine as K=5
    augmented fp32 matmuls (d2 = |p|^2 - 2 p.g + |g|^2 is linear in
    augmented features), 4x row-tiled via tile_position since K=5.
  - The min-reduction over N^2 entries is the bottleneck. A custom DVE op
    (min(Src0,Src1) with MIN-accumulate) retires TWO distance entries per
    DVE cycle: the PE emits two half-matrices A (opposite points 0..4095)
    and B (4096..8191); ScalarE copies B from PSUM to SBUF (1 elem/cyc,
    in parallel), and the DVE op reads A from PSUM + B from SBUF,
    producing the running row-min. fp32 throughout (cancellation in
    p^2-2pg+g^2 needs it; matches the fp32 reference).
"""

import os
import sys

import numpy as np

for _p in ("/opt/trn_rl_repo", "/root/.axon_site/_ro/trn_rl_repo"):
    if os.path.isdir(_p) and _p not in sys.path:
        sys.path.insert(0, _p)

import concourse.bacc as bacc
import concourse.tile as tile
from concourse import dve_ops as _dve_ops
from concourse import mybir
from concourse.bass_utils import run_bass_kernel_spmd
from concourse.dve_spec import AluOp, C0, Spec, Src0, Src1, minn
from concourse.dve_spec import lower as _dve_lower
from concourse.dve_table_gen import dve_ver_for
from concourse.dve_uop import DveOpSpec

F32 = mybir.dt.float32
P = 128          # partitions / points per row tile
K = 5            # augmented feature dim
B = 4
N = 8192
HALF = N // 2    # points per core (4096)
NRT = HALF // P  # row tiles per direction (32)
CHUNK = 1024     # opposite-point columns (per half) per pipeline iteration
NCHUNK = HALF // CHUNK
BIG = 1.0e30


def _ref_min2(in0, in1, c0, c1, c2):
    b = np.minimum(np.asarray(in0, np.float32), np.asarray(in1, np.float32))
    acc = np.minimum(b.reshape(b.shape[0], -1).min(-1, keepdims=True), c0)
    return b, acc


def _get_min2_op():
    """Register (once) a custom DVE op: out = min(in0, in1),
    accum_out = min(s0, min over free dim of out)."""
    name = "MIN2_REDUCE_ANT"
    for op in _dve_ops.OPS:
        if op.name == name:
            return op
    spec = Spec(body=minn(Src0, Src1), accum=AluOp.MIN, accum_init=C0,
                reference=_ref_min2)
    row = max(_dve_ops._SUB_OPCODE_FOR_NAME.values()) + 1
    assert row < 0x20
    _dve_ops._SUB_OPCODE_FOR_NAME[name] = row
    shas = {}
    for ver in ("v3", "v4"):
        try:
            uops = _dve_lower(spec, ver=ver)
        except Exception:
            continue
        shas[ver] = DveOpSpec(name=name, opcode=row, uops=uops,
                              rd1_en=True).sha(ver)
    op = _dve_ops.DveOp(name, spec, subdim=False, uops_sha=shas)
    _dve_ops.OPS.append(op)
    _dve_ops.CUSTOM_DVE_SPECS[name] = spec
    return op


def _build_nc(nrt=NRT, nchunk=NCHUNK, half=HALF):
    """Build the per-core Bass program (SPMD across 8 cores)."""
    min2 = _get_min2_op()
    nc = bacc.Bacc(None)

    wf = [nc.dram_tensor(f"w_feat_{d}", [K, nrt * P], F32, kind="ExternalInput")
          for d in (0, 1)]
    ra = [nc.dram_tensor(f"r_a_{d}", [K, half], F32, kind="ExternalInput")
          for d in (0, 1)]
    rb = [nc.dram_tensor(f"r_b_{d}", [K, half], F32, kind="ExternalInput")
          for d in (0, 1)]
    wv = nc.dram_tensor("wvec", [P, nrt], F32, kind="ExternalInput")
    out = nc.dram_tensor("out", [P, 1], F32, kind="ExternalOutput")

    with tile.TileContext(nc) as tc:
        with (
            tc.tile_pool(name="const", bufs=1) as cpool,
            tc.tile_pool(name="work", bufs=4) as wpool,
            tc.tile_pool(name="psum", bufs=2, space="PSUM") as ppool,
            tc.tile_pool(name="stats", bufs=1) as spool,
        ):
            # Stationary (lhsT) features replicated into all 4 PE row-group
            # quadrants; moving (rhs) features: half-A in quadrants 0/1,
            # half-B in quadrants 2/3.
            w_t = []
            r_t = []
            for d in (0, 1):
                wt = cpool.tile([P, nrt * P], F32, name=f"w_t_{d}")
                for q in (0, 32, 64, 96):
                    nc.sync.dma_start(wt[q:q + K, :], wf[d][:, :])
                w_t.append(wt)
                rt_ = cpool.tile([P, half], F32, name=f"r_t_{d}")
                for q, src in ((0, ra[d]), (32, ra[d]), (64, rb[d]), (96, rb[d])):
                    nc.sync.dma_start(rt_[q:q + K, :], src[:, :])
                r_t.append(rt_)

            wv_t = cpool.tile([P, nrt], F32, name="wv_t")
            nc.sync.dma_start(wv_t[:, :], wv[:, :])

            # one partial-min column per (direction, row-tile, chunk)
            acc = spool.tile([P, 2 * nrt * nchunk], F32, name="acc")

            for d in (0, 1):
                for r in range(nrt):
                    rsl = slice(r * P, (r + 1) * P)
                    for c in range(nchunk):
                        col = (d * nrt + r) * nchunk + c
                        b0 = c * CHUNK
                        a_ps = ppool.tile([P, CHUNK], F32, name="a_ps")
                        b_ps = ppool.tile([P, CHUNK], F32, name="b_ps")
                        nc.tensor.matmul(
                            b_ps[:, 0:512], w_t[d][64:64 + K, rsl],
                            r_t[d][64:64 + K, b0:b0 + 512],
                            tile_position=(64, 0))
                        nc.tensor.matmul(
                            b_ps[:, 512:1024], w_t[d][96:96 + K, rsl],
                            r_t[d][96:96 + K, b0 + 512:b0 + 1024],
                            tile_position=(96, 0))
                        nc.tensor.matmul(
                            a_ps[:, 0:512], w_t[d][0:K, rsl],
                            r_t[d][0:K, b0:b0 + 512], tile_position=(0, 0))
                        nc.tensor.matmul(
                            a_ps[:, 512:1024], w_t[d][32:32 + K, rsl],
                            r_t[d][32:32 + K, b0 + 512:b0 + 1024],
                            tile_position=(32, 0))

                        b_sb = wpool.tile([P, CHUNK], F32, name="b_sb")
                        nc.scalar.copy(b_sb, b_ps)

                        scr = wpool.tile([P, CHUNK], F32, name="scr")
                        nc.vector._custom_dve(
                            min2, out=scr, in0=a_ps, in1=b_sb,
                            s0=BIG, accum_out=acc[:, col:col + 1])

            # reduce chunk partials: [P, 2*nrt, nchunk] -min-> [P, 2*nrt]
            accr = spool.tile([P, 2 * nrt], F32, name="accr")
            nc.vector.tensor_reduce(
                accr, acc.rearrange("p (a c) -> p a c", c=nchunk),
                axis=mybir.AxisListType.X, op=mybir.AluOpType.min)

            # tail: out[p] = sum_r (acc0[p,r] + acc1[p,r]) * wvec[p,r]
            both = spool.tile([P, nrt], F32, name="both")
            nc.vector.tensor_tensor(both, accr[:, 0:nrt], accr[:, nrt:2 * nrt],
                                    op=mybir.AluOpType.add)
            prod = spool.tile([P, nrt], F32, name="prod")
            nc.vector.tensor_tensor(prod, both, wv_t, op=mybir.AluOpType.mult)
            fin = spool.tile([P, 1], F32, name="fin")
            nc.vector.tensor_reduce(fin, prod, axis=mybir.AxisListType.X,
                                    op=mybir.AluOpType.add)
            nc.sync.dma_start(out[:, :], fin[:, :])

    return nc


def _wfeat(sel):
    """[M,3] -> [5,M] stationary features [x, y, z, |p|^2, 1]."""
    x = sel.astype(np.float64)
    sq = (x * x).sum(-1)
    return np.ascontiguousarray(
        np.stack([x[:, 0], x[:, 1], x[:, 2], sq, np.ones(len(x))], 0)
    ).astype(np.float32)


def _sfeat(sel):
    """[M,3] -> [5,M] moving features [-2x, -2y, -2z, 1, |g|^2]."""
    x = sel.astype(np.float64)
    sq = (x * x).sum(-1)
    return np.ascontiguousarray(
        np.stack([-2 * x[:, 0], -2 * x[:, 1], -2 * x[:, 2],
                  np.ones(len(x)), sq], 0)
    ).astype(np.float32)


def _make_in_maps(inputs, targets, weight):
    in_maps = []
    for core in range(8):
        b, h = core // 2, core % 2
        lo, hi = h * HALF, (h + 1) * HALF
        pred = np.asarray(inputs[b], dtype=np.float32)
        gt = np.asarray(targets[b], dtype=np.float32)
        wvec = (np.asarray(weight[b], dtype=np.float32)[lo:hi]
                .astype(np.float64).mean(-1).reshape(NRT, P).T)
        in_maps.append({
            "w_feat_0": _wfeat(pred[lo:hi]),
            "w_feat_1": _wfeat(gt[lo:hi]),
            "r_a_0": _sfeat(gt[:HALF]),
            "r_b_0": _sfeat(gt[HALF:]),
            "r_a_1": _sfeat(pred[:HALF]),
            "r_b_1": _sfeat(pred[HALF:]),
            "wvec": np.ascontiguousarray(wvec).astype(np.float32),
        })
    return in_maps


_NC_CACHE = {}


def _get_nc():
    if "nc" not in _NC_CACHE:
        nc = _build_nc()
        nc.finalize()  # Bacc: run compile passes (regalloc, event-sem split)
        _NC_CACHE["nc"] = nc
    return _NC_CACHE["nc"]


def _run(inputs, targets, weight, trace=False, **kw):
    nc = _get_nc()
    in_maps = _make_in_maps(inputs, targets, weight)
    res = run_bass_kernel_spmd(nc, in_maps, list(range(8)), trace=trace, **kw)
    total = 0.0
    for r in res.results:
        total += np.asarray(r["out"], dtype=np.float64).sum()
    loss = total / (B * N)
    return np.float32(loss), res


def kernel(inputs, targets, weight):
    loss, _ = _run(inputs, targets, weight)
    return loss


if __name__ == "__main__":
    rng = np.random.default_rng(0)
    ins = {
        "inputs": rng.standard_normal((B, N, 3), dtype=np.float32),
        "targets": rng.standard_normal((B, N, 3), dtype=np.float32),
        "weight": rng.random((B, N, 3), dtype=np.float32),
    }
    got = kernel(**ins)

    w = ins["weight"].mean(-1)
    want = 0.0
    for b in range(B):
        p = ins["inputs"][b].astype(np.float64)
        g = ins["targets"][b].astype(np.float64)
        d2 = ((p[:, None, :] - g[None, :, :]) ** 2).sum(-1)
        want += ((d2.min(1) + d2.min(0)) * w[b]).sum()
    want /= B * N
    print("kernel:", got, "ref:", want, "rel:", abs(got - want) / abs(want))


# revision 9
# speedup vs baseline: 3400.1263x; 3400.1263x over previous
"""Trainium2 Bass kernel for weighted Chamfer loss.

Problem: B=4 batches of N=8192 3-D points (pred, gt) + per-point weights.
loss = mean_{b,n}[ (min_m d2(p_n,g_m) + min_m d2(g_n,p_m)) * mean(weight) ]

Strategy (8 NeuronCores):
  - Shard (batch, half): core c -> batch c//2, point-half c%2. Each core
    computes, for its 4096 points, the exact min squared distance to all
    8192 opposite points, in BOTH directions (pred->gt for its pred half,
    gt->pred for its gt half). The host just sums the 8 per-core weighted
    partial sums -- no cross-core combining.
  - On-core: distance matrices come from the TensorEngine as K=5
    augmented fp32 matmuls (d2 = |p|^2 - 2 p.g + |g|^2 is linear in
    augmented features), 4x row-tiled via tile_position since K=5.
  - The min-reduction over N^2 entries is the bottleneck. A custom DVE op
    (min(Src0,Src1) with MIN-accumulate) retires TWO distance entries per
    DVE cycle: the PE emits two half-matrices A (opposite points 0..4095)
    and B (4096..8191); ScalarE copies B from PSUM to SBUF (1 elem/cyc,
    in parallel), and the DVE op reads A from PSUM + B from SBUF,
    producing the running row-min. fp32 throughout (cancellation in
    p^2-2pg+g^2 needs it; matches the fp32 reference).
"""

import os
import sys

import numpy as np

for _p in ("/opt/trn_rl_repo", "/root/.axon_site/_ro/trn_rl_repo"):
    if os.path.isdir(_p) and _p not in sys.path:
        sys.path.insert(0, _p)

import concourse.bacc as bacc
import concourse.tile as tile
from concourse import dve_ops as _dve_ops
from concourse import mybir
from concourse.bass_utils import run_bass_kernel_spmd
from concourse.dve_spec import AluOp, C0, Spec, Src0, Src1, minn
from concourse.dve_spec import lower as _dve_lower
from concourse.dve_table_gen import dve_ver_for
from concourse.dve_uop import DveOpSpec

F32 = mybir.dt.float32
P = 128          # partitions / points per row tile
K = 5            # augmented feature dim
B = 4
N = 8192
HALF = N // 2    # points per core (4096)
NRT = HALF // P  # row tiles per direction (32)
CHUNK = 1024     # opposite-point columns (per half) per pipeline iteration
NCHUNK = HALF // CHUNK
BIG = 1.0e30


def _ref_min2(in0, in1, c0, c1, c2):
    b = np.minimum(np.asarray(in0, np.float32), np.asarray(in1, np.float32))
    acc = np.minimum(b.reshape(b.shape[0], -1).min(-1, keepdims=True), c0)
    return b, acc


def _get_min2_op():
    """Register (once) a custom DVE op: out = min(in0, in1),
    accum_out = min(s0, min over free dim of out)."""
    name = "MIN2_REDUCE_ANT"
    for op in _dve_ops.OPS:
        if op.name == name:
            return op
    spec = Spec(body=minn(Src0, Src1), accum=AluOp.MIN, accum_init=C0,
                reference=_ref_min2)
    row = max(_dve_ops._SUB_OPCODE_FOR_NAME.values()) + 1
    assert row < 0x20
    _dve_ops._SUB_OPCODE_FOR_NAME[name] = row
    shas = {}
    for ver in ("v3", "v4"):
        try:
            uops = _dve_lower(spec, ver=ver)
        except Exception:
            continue
        shas[ver] = DveOpSpec(name=name, opcode=row, uops=uops,
                              rd1_en=True).sha(ver)
    op = _dve_ops.DveOp(name, spec, subdim=False, uops_sha=shas)
    _dve_ops.OPS.append(op)
    _dve_ops.CUSTOM_DVE_SPECS[name] = spec
    return op


def _build_nc(nrt=NRT, nchunk=NCHUNK, half=HALF):
    """Build the per-core Bass program (SPMD across 8 cores)."""
    min2 = _get_min2_op()
    nc = bacc.Bacc(None)

    wf = [nc.dram_tensor(f"w_feat_{d}", [K, nrt * P], F32, kind="ExternalInput")
          for d in (0, 1)]
    ra = [nc.dram_tensor(f"r_a_{d}", [K, half], F32, kind="ExternalInput")
          for d in (0, 1)]
    rb = [nc.dram_tensor(f"r_b_{d}", [K, half], F32, kind="ExternalInput")
          for d in (0, 1)]
    wv = nc.dram_tensor("wvec", [P, nrt], F32, kind="ExternalInput")
    out = nc.dram_tensor("out", [P, 1], F32, kind="ExternalOutput")

    with tile.TileContext(nc) as tc:
        with (
            tc.tile_pool(name="const", bufs=1) as cpool,
            tc.tile_pool(name="work", bufs=4) as wpool,
            tc.tile_pool(name="psum", bufs=2, space="PSUM") as ppool,
            tc.tile_pool(name="stats", bufs=1) as spool,
        ):
            # Stationary (lhsT) features replicated into all 4 PE row-group
            # quadrants; moving (rhs) features: half-A in quadrants 0/1,
            # half-B in quadrants 2/3.
            w_t = []
            r_t = []
            for d in (0, 1):
                wt = cpool.tile([P, nrt * P], F32, name=f"w_t_{d}")
                for q in (0, 32, 64, 96):
                    nc.sync.dma_start(wt[q:q + K, :], wf[d][:, :])
                w_t.append(wt)
                rt_ = cpool.tile([P, half], F32, name=f"r_t_{d}")
                for q, src in ((0, ra[d]), (32, ra[d]), (64, rb[d]), (96, rb[d])):
                    nc.sync.dma_start(rt_[q:q + K, :], src[:, :])
                r_t.append(rt_)

            wv_t = cpool.tile([P, nrt], F32, name="wv_t")
            nc.sync.dma_start(wv_t[:, :], wv[:, :])

            # one partial-min column per (direction, row-tile, chunk)
            acc = spool.tile([P, 2 * nrt * nchunk], F32, name="acc")

            for d in (0, 1):
                for r in range(nrt):
                    rsl = slice(r * P, (r + 1) * P)
                    for c in range(nchunk):
                        col = (d * nrt + r) * nchunk + c
                        b0 = c * CHUNK
                        a_ps = ppool.tile([P, CHUNK], F32, name="a_ps")
                        b_ps = ppool.tile([P, CHUNK], F32, name="b_ps")
                        nc.tensor.matmul(
                            b_ps[:, 0:512], w_t[d][64:64 + K, rsl],
                            r_t[d][64:64 + K, b0:b0 + 512],
                            tile_position=(64, 0))
                        nc.tensor.matmul(
                            b_ps[:, 512:1024], w_t[d][96:96 + K, rsl],
                            r_t[d][96:96 + K, b0 + 512:b0 + 1024],
                            tile_position=(96, 0))
                        nc.tensor.matmul(
                            a_ps[:, 0:512], w_t[d][0:K, rsl],
                            r_t[d][0:K, b0:b0 + 512], tile_position=(0, 0))
                        nc.tensor.matmul(
                            a_ps[:, 512:1024], w_t[d][32:32 + K, rsl],
                            r_t[d][32:32 + K, b0 + 512:b0 + 1024],
                            tile_position=(32, 0))

                        b_sb = wpool.tile([P, CHUNK], F32, name="b_sb")
                        nc.scalar.copy(b_sb, b_ps)

                        scr = wpool.tile([P, CHUNK], F32, name="scr")
                        nc.vector._custom_dve(
                            min2, out=scr, in0=a_ps, in1=b_sb,
                            s0=BIG, accum_out=acc[:, col:col + 1])

            # reduce chunk partials: [P, 2*nrt, nchunk] -min-> [P, 2*nrt]
            accr = spool.tile([P, 2 * nrt], F32, name="accr")
            nc.vector.tensor_reduce(
                accr, acc.rearrange("p (a c) -> p a c", c=nchunk),
                axis=mybir.AxisListType.X, op=mybir.AluOpType.min)

            # tail: out[p] = sum_r (acc0[p,r] + acc1[p,r]) * wvec[p,r]
            both = spool.tile([P, nrt], F32, name="both")
            nc.vector.tensor_tensor(both, accr[:, 0:nrt], accr[:, nrt:2 * nrt],
                                    op=mybir.AluOpType.add)
            prod = spool.tile([P, nrt], F32, name="prod")
            nc.vector.tensor_tensor(prod, both, wv_t, op=mybir.AluOpType.mult)
            fin = spool.tile([P, 1], F32, name="fin")
            nc.vector.tensor_reduce(fin, prod, axis=mybir.AxisListType.X,
                                    op=mybir.AluOpType.add)
            nc.sync.dma_start(out[:, :], fin[:, :])

    return nc


def _wfeat(sel):
    """[M,3] -> [5,M] stationary features [x, y, z, |p|^2, 1]."""
    x = sel.astype(np.float64)
    sq = (x * x).sum(-1)
    return np.ascontiguousarray(
        np.stack([x[:, 0], x[:, 1], x[:, 2], sq, np.ones(len(x))], 0)
    ).astype(np.float32)


def _sfeat(sel):
    """[M,3] -> [5,M] moving features [-2x, -2y, -2z, 1, |g|^2]."""
    x = sel.astype(np.float64)
    sq = (x * x).sum(-1)
    return np.ascontiguousarray(
        np.stack([-2 * x[:, 0], -2 * x[:, 1], -2 * x[:, 2],
                  np.ones(len(x)), sq], 0)
    ).astype(np.float32)


def _make_in_maps(inputs, targets, weight):
    in_maps = []
    for core in range(8):
        b, h = core // 2, core % 2
        lo, hi = h * HALF, (h + 1) * HALF
        pred = np.asarray(inputs[b], dtype=np.float32)
        gt = np.asarray(targets[b], dtype=np.float32)
        wvec = (np.asarray(weight[b], dtype=np.float32)[lo:hi]
                .astype(np.float64).mean(-1).reshape(NRT, P).T)
        in_maps.append({
            "w_feat_0": _wfeat(pred[lo:hi]),
            "w_feat_1": _wfeat(gt[lo:hi]),
            "r_a_0": _sfeat(gt[:HALF]),
            "r_b_0": _sfeat(gt[HALF:]),
            "r_a_1": _sfeat(pred[:HALF]),
            "r_b_1": _sfeat(pred[HALF:]),
            "wvec": np.ascontiguousarray(wvec).astype(np.float32),
        })
    return in_maps


_NC_CACHE = {}


def _get_nc():
    if "nc" not in _NC_CACHE:
        nc = _build_nc()
        nc.finalize()  # Bacc: run compile passes (regalloc, event-sem split)
        _NC_CACHE["nc"] = nc
    return _NC_CACHE["nc"]


def _make_runner(nc):
    """Jitted SPMD executor for a finalized Bass module (same execution
    path run_bass_kernel_spmd takes under axon -- bass2jax's _bass_exec_p
    via shard_map -- but built once so repeat calls don't re-jit)."""
    import jax
    from jax.experimental.shard_map import shard_map
    from jax.sharding import Mesh, PartitionSpec

    from concourse import bass2jax

    bass2jax.install_neuronx_cc_hook()
    n_cores = 8
    pname = nc.partition_id_tensor.name if nc.partition_id_tensor else None
    in_names, out_names, out_avals, zero_shapes = [], [], [], []
    for alloc in nc.m.functions[0].allocations:
        if not isinstance(alloc, mybir.MemoryLocationSet):
            continue
        name = alloc.memorylocations[0].name
        if alloc.kind == "ExternalInput":
            if name != pname:
                in_names.append(name)
        elif alloc.kind == "ExternalOutput":
            out_names.append(name)
            shape, dt = tuple(alloc.tensor_shape), mybir.dt.np(alloc.dtype)
            out_avals.append(jax.core.ShapedArray(shape, dt))
            zero_shapes.append((shape, dt))
    n_params, n_outs = len(in_names), len(out_names)
    all_names = [*in_names, *out_names] + ([pname] if pname else [])
    donate = tuple(range(n_params, n_params + n_outs))

    def _body(*args):
        operands = list(args)
        if pname is not None:
            operands.append(bass2jax.partition_id_tensor())
        return tuple(bass2jax._bass_exec_p.bind(
            *operands,
            out_avals=tuple(out_avals),
            in_names=tuple(all_names),
            out_names=tuple(out_names),
            lowering_input_output_aliases=(),
            sim_require_finite=True,
            sim_require_nnan=True,
            nc=nc,
        ))

    devices = jax.devices()[:n_cores]
    mesh = Mesh(np.asarray(devices), ("core",))
    sharded = jax.jit(
        shard_map(_body, mesh=mesh,
                  in_specs=(PartitionSpec("core"),) * (n_params + n_outs),
                  out_specs=(PartitionSpec("core"),) * n_outs,
                  check_rep=False),
        donate_argnums=donate, keep_unused=True)
    return {"sharded": sharded, "mesh": mesh, "in_names": in_names,
            "out_names": out_names, "zero_shapes": zero_shapes,
            "n_cores": n_cores}


def _get_runner():
    if "runner" not in _NC_CACHE:
        _NC_CACHE["runner"] = _make_runner(_get_nc())
    return _NC_CACHE["runner"]


def _run_maps(in_maps):
    """Execute the cached runner on per-core input maps; returns list of
    per-core output dicts."""
    r = _get_runner()
    n_cores = r["n_cores"]
    concat_in = [
        np.concatenate([np.asarray(in_maps[c][nm]) for c in range(n_cores)],
                       axis=0)
        for nm in r["in_names"]
    ]
    concat_zeros = [np.zeros((n_cores * s[0], *s[1:]), dt)
                    for (s, dt) in r["zero_shapes"]]
    out_arrs = [np.asarray(a) for a in r["sharded"](*concat_in, *concat_zeros)]
    return [
        {nm: out_arrs[i].reshape(n_cores, -1, *out_arrs[i].shape[1:])[c]
         for i, nm in enumerate(r["out_names"])}
        for c in range(n_cores)
    ]


def _finish(results):
    total = 0.0
    for r in results:
        total += np.asarray(r["out"], dtype=np.float64).sum()
    return np.float32(total / (B * N))


def _run(inputs, targets, weight, trace=False, **kw):
    """run_bass_kernel_spmd path (kept for tracing/debug)."""
    nc = _get_nc()
    in_maps = _make_in_maps(inputs, targets, weight)
    res = run_bass_kernel_spmd(nc, in_maps, list(range(8)), trace=trace, **kw)
    return _finish(res.results), res


def kernel(inputs, targets, weight):
    in_maps = _make_in_maps(inputs, targets, weight)
    try:
        return _finish(_run_maps(in_maps))
    except Exception:
        loss, _ = _run(inputs, targets, weight)
        return loss


if __name__ == "__main__":
    rng = np.random.default_rng(0)
    ins = {
        "inputs": rng.standard_normal((B, N, 3), dtype=np.float32),
        "targets": rng.standard_normal((B, N, 3), dtype=np.float32),
        "weight": rng.random((B, N, 3), dtype=np.float32),
    }
    got = kernel(**ins)

    w = ins["weight"].mean(-1)
    want = 0.0
    for b in range(B):
        p = ins["inputs"][b].astype(np.float64)
        g = ins["targets"][b].astype(np.float64)
        d2 = ((p[:, None, :] - g[None, :, :]) ** 2).sum(-1)
        want += ((d2.min(1) + d2.min(0)) * w[b]).sum()
    want /= B * N
    print("kernel:", got, "ref:", want, "rel:", abs(got - want) / abs(want))
